# revision 47
# baseline (speedup 1.0000x reference)
"""TRN2 Bass kernel for nn_BimodalAttention.

Reference computation (B=16, T=2048, D1=D2=1024, U=1024):
    f1 = X1 @ W1 + b1 ; f2 = X2 @ W2 + b2
    H  = tanh(concat(f1, f2) @ W + b)            # [B,T,U]
    s  = H @ c ; a = softmax(s, axis=T)          # [B,T,1]
    out[b] = sum_t a[b,t] * H[b,t]               # [B,U]

Device strategy (data-parallel over batch, 2 batches per core, 8 cores):
  * Host folds the linear chain: M1 = W1 @ W[:U], M2 = W2 @ W[U:], so the
    device computes H = tanh(Xcat @ M + beff) with M = [M1; M2] — half the
    matmul FLOPs of the literal graph.
  * Host pre-transposes/tiles Xcat to [B, K/128, 128, T] so every lhsT tile
    DMA is contiguous, and replicates the context vector across the 128
    partitions so scores are row-local DVE work.
  * Main matmuls run as float32r (full PE rate; measured steady cadence
    ~227ns per 512-col matmul = stream time + ~15ns, i.e. at the hw
    floor).  tanh writes H as f32r directly (the ACT engine rounds at
    write), so the weighted-sum matmuls consume it with no cast; the
    score path reads the same bytes through an f32 bitcast.
  * PE warm-up + deterministic dummy fillers (on memset data, no DMA
    dependency) cover the ~8us DMA queue-startup window and the
    DMA-saturated weight-load phase; chunk 0's uh1 sweeps are k-quarter
    interleaved across the 4 t-tiles (4 open PSUM accumulations) so each
    weight quarter's arrival feeds a full PE group.
  * Softmax over T: no max-subtraction (scores are ~N(0,11) by
    construction; exp overflows only past 88) — a clamp at 60 guards
    against inf.  The weighted time-sum uses the unnormalized exp weights
    (PE matmuls, 1-tile pipeline slack); normalization by 1/Z happens on
    the HOST (kernel ships the raw weighted sum and the per-partition exp
    sums), which shortens the device tail and frees PSUM banks so both
    batches' o_ps can overlap.
"""
import numpy as np

import concourse.bacc as bacc
import concourse.mybir as mybir
from concourse.bass_utils import run_bass_kernel_spmd
from concourse.tile import TileContext

F32 = mybir.dt.float32
F32R = mybir.dt.float32r

N_CORES = 8
B, T, D, UNITS = 16, 2048, 1024, 1024
KD = 2 * D          # folded contraction dim (seq1 ++ seq2)
BPC = B // N_CORES  # batches per core

_NC_CACHE = {}


def build_nc(bpc=BPC, t=T, kd=KD, units=UNITS, has_bias=False, tchunk=512,
             n_warm=44, n_fill=6):
    """Build the per-core Bass module (same program on all cores)."""
    nc = bacc.Bacc(None, target_bir_lowering=False)

    nk = kd // 128              # k-blocks in contraction
    nt = t // 128               # t-blocks
    nuh = (units + 511) // 512  # 512-wide u column groups
    uh_w = units // nuh
    ntc = t // tchunk           # streamed X chunks per batch
    tpc = tchunk // 128         # t-blocks per chunk

    xt = nc.declare_dram_parameter("xt", [bpc, nk, 128, t], F32R, isOutput=False)
    mw = nc.declare_dram_parameter("mw", [nk, 128, units], F32R, isOutput=False)
    crep = nc.declare_dram_parameter("crep", [128, units], F32, isOutput=False)
    if has_bias:
        brep = nc.declare_dram_parameter("brep", [128, units], F32,
                                         isOutput=False)
    o_raw = nc.declare_dram_parameter("o_raw", [bpc, units], F32, isOutput=True)
    esums = nc.declare_dram_parameter("esums", [bpc, 128, 1], F32,
                                      isOutput=True)

    with TileContext(nc) as tc:
        with (
            tc.tile_pool(name="wpool", bufs=1) as wpool,
            tc.tile_pool(name="xpool", bufs=2) as xpool,
            tc.tile_pool(name="hpool", bufs=tpc + 2) as hpool,
            tc.tile_pool(name="spool", bufs=2) as spool,
            tc.tile_pool(name="scratch", bufs=2) as scratch,
            tc.tile_pool(name="mainps", bufs=4, space="PSUM") as mainps,
            tc.tile_pool(name="outps", bufs=2, space="PSUM") as outps,
        ):
            # ---- resident small tensors -------------------------------
            mwt = wpool.tile([128, nk * units], F32R, name="mwt")
            mwt4 = mwt.rearrange("p (k h u) -> p k h u", k=nk, h=nuh)
            kq = max(1, nk // 4)
            mw_r = mw.rearrange("k p u -> p k u")
            crep_s = wpool.tile([128, units], F32, name="crep_s")
            if has_bias:
                brep_s = wpool.tile([128, units], F32, name="brep_s")
                nc.sync.dma_start(out=brep_s[:, :], in_=brep[:, :])

            # ones column (f32r) for the final cross-partition reduction
            # of the ACT/DVE-accumulated weighted sum
            ones_f = wpool.tile([128, 1], F32, name="ones_f")
            ones_r = wpool.tile([128, 1], F32R, name="ones_r")
            nc.vector.memset(ones_f[:, :], 1.0)
            nc.vector.tensor_copy(ones_r[:, :], ones_f[:, :])

            first_deferred = True
            for b in range(bpc):
                s_all = spool.tile([128, nt], F32, tag="s_all", name="s_all")
                s_c = spool.tile([128, nt], F32, tag="s_c", name="s_c")
                e_f32 = spool.tile([128, nt], F32, tag="e_f32", name="e_f32")
                e_r = spool.tile([128, nt], F32R, tag="e_r", name="e_r")
                o_ps = outps.tile([1, units], F32, tag="o_ps", name="o_ps")
                # weighted-sum accumulator: tiles 0..nt-3 are accumulated
                # OFF the PE (ACT per-partition scale-mul + DVE add), so
                # only the last two tiles' weighted sums plus one final
                # ones-vector reduction run as matmuls -> ~7us less
                # PE-critical-path work per batch.  The last 'acc' tile's
                # add writes the f32r copy the final matmul needs.
                acc0 = spool.tile([128, units], F32, tag="acc", name="acc0")
                acc1 = spool.tile([128, units], F32, tag="acc", name="acc1")
                accs = [acc0, acc1]
                acc_r = spool.tile([128, units], F32R, tag="acc_r",
                                   name="acc_r", bufs=1)
                wsum_pending = []

                def emit_wacc(t0, h_t):
                    """Fold tile t0 into the weighted sum: one fused DVE
                    op acc' = (H * e_col) + acc (ping-pong accumulators,
                    no same-instruction aliasing)."""
                    e_col = e_f32[:, t0:t0 + 1]
                    h_f = h_t.bitcast(F32)
                    if t0 >= nt - 2:
                        # last two tiles: PE path (keeps the end-of-batch
                        # chain short: exp -> 2 matmuls).  Tile nt-2 OPENS
                        # the o_ps accumulation group (its e is ready well
                        # before the DVE acc_r copy finishes); the
                        # cross-partition acc reduction follows it so the
                        # PE never stalls on the copy.
                        for wuh in range(nuh):
                            nc.tensor.matmul(
                                out=o_ps[0:1, wuh * uh_w:(wuh + 1) * uh_w],
                                lhsT=e_r[:, t0:t0 + 1],
                                rhs=h_t[:, wuh * uh_w:(wuh + 1) * uh_w],
                                start=(t0 == nt - 2),
                                stop=(t0 == nt - 1),
                            )
                        if t0 == nt - 2:
                            for wuh in range(nuh):
                                nc.tensor.matmul(
                                    out=o_ps[0:1,
                                             wuh * uh_w:(wuh + 1) * uh_w],
                                    lhsT=ones_r[:, 0:1],
                                    rhs=acc_r[:, wuh * uh_w:(wuh + 1) * uh_w],
                                    start=False, stop=False,
                                )
                        return
                    dst, src = accs[t0 % 2], accs[1 - t0 % 2]
                    if t0 == 0:
                        nc.vector.tensor_scalar_mul(dst[:, :], h_f[:, :],
                                                    e_col)
                    else:
                        nc.vector.scalar_tensor_tensor(
                            out=dst[:, :], in0=h_f[:, :], scalar=e_col,
                            in1=src[:, :],
                            op0=mybir.AluOpType.mult,
                            op1=mybir.AluOpType.add,
                        )
                    if t0 == nt - 3:
                        nc.vector.tensor_copy(acc_r[:, :], dst[:, :])

                if b == 0:
                    # PE warm-up: dummy matmuls into o_ps (the real t0=0
                    # weighted-sum matmul re-clears it with start=True).
                    # Gets HAM to K=8/8 while the first loads stream in.
                    # The warm data is produced on-chip (memset + DVE cast
                    # - a legal f32r producer) so the warm-up has no DMA
                    # dependency and starts ~1us in, covering the DMA
                    # queue-startup dead window.
                    warm_f = wpool.tile([128, uh_w], F32, name="warm_f")
                    warm = wpool.tile([128, uh_w], F32R, name="warm")
                    nc.vector.memset(warm_f[:, :], 0.0)
                    nc.vector.tensor_copy(warm[:, :], warm_f[:, :])
                    for _ in range(n_warm):
                        nc.tensor.matmul(
                            out=o_ps[0:1, 0:uh_w],
                            lhsT=warm[:, 0:1], rhs=warm[:, 0:uh_w],
                            start=True, stop=True,
                        )

                for tcix in range(ntc):
                    first_chunk = first_deferred
                    a_t = xpool.tile([128, nk * tchunk], F32R, tag="a_t",
                                     name="a_t")
                    a_t3 = a_t.rearrange("p (k w) -> p k w", k=nk)
                    x_src = xt[b].rearrange("k p w -> p k w")[
                        :, :, tcix * tchunk:(tcix + 1) * tchunk]
                    if first_chunk:
                        # context vector first (the score chain needs it
                        # from the very first tile), then interleaved
                        # k-quarter loads of mw[uh0] and chunk 0 so the
                        # uh0 tiles start after ~2MB; uh1 quarters follow
                        # (consumed by the later uh1 pass of chunk 0).
                        nc.sync.dma_start(out=crep_s[:, :], in_=crep[:, :])
                        for q in range(0, nk, kq):
                            nc.sync.dma_start(
                                out=mwt4[:, q:q + kq, 0, :],
                                in_=mw_r[:, q:q + kq, 0:uh_w],
                            )
                            nc.sync.dma_start(
                                out=a_t3[:, q:q + kq, :],
                                in_=x_src[:, q:q + kq, :],
                            )
                        for uh in range(1, nuh):
                            for q in range(0, nk, kq):
                                nc.sync.dma_start(
                                    out=mwt4[:, q:q + kq, uh, :],
                                    in_=mw_r[:, q:q + kq,
                                             uh * uh_w:(uh + 1) * uh_w],
                                )
                        first_deferred = False
                    else:
                        # halves: finer arrival granularity so the first
                        # tiles of a late chunk start on the first half
                        half = tchunk // 2
                        nc.sync.dma_start(out=a_t3[:, :, 0:half],
                                          in_=x_src[:, :, 0:half])
                        nc.sync.dma_start(out=a_t3[:, :, half:],
                                          in_=x_src[:, :, half:])

                    h_ts = {}

                    def half_matmuls(i, uh, ks, ps):
                        for k in ks:
                            nc.tensor.matmul(
                                out=ps[:, :],
                                lhsT=a_t[:, k * tchunk + i * 128:
                                         k * tchunk + (i + 1) * 128],
                                rhs=mwt[:, k * units + uh * uh_w:
                                        k * units + (uh + 1) * uh_w],
                                start=(k == 0),
                                stop=(k == nk - 1),
                            )

                    def half_epilogue(i, uh, ps):
                        t0 = tcix * tpc + i
                        h_t = h_ts[i]
                        uhs = slice(uh * uh_w, (uh + 1) * uh_w)
                        if has_bias:
                            nc.vector.tensor_tensor(
                                out=ps[:, :], in0=ps[:, :],
                                in1=brep_s[:, uhs],
                                op=mybir.AluOpType.add,
                            )
                        # tanh writes the f32r H half directly (the ACT
                        # engine rounds at write) — no separate cast
                        nc.scalar.activation(
                            out=h_t[:, uhs],
                            in_=ps[:, :],
                            func=mybir.ActivationFunctionType.Tanh,
                        )
                        # score partial in ONE fused DVE op:
                        # junk = (h*1.0) \odot c, accum_out = row-sum
                        junk = scratch.tile([128, uh_w], F32,
                                            tag="junk", name="junk",
                                            bufs=3)
                        nc.vector.scalar_tensor_tensor(
                            out=junk[:, :],
                            in0=h_t.bitcast(F32)[:, uhs],
                            scalar=1.0,
                            in1=crep_s[:, uhs],
                            op0=mybir.AluOpType.mult,
                            op1=mybir.AluOpType.mult,
                            accum_out=(s_c if uh == 0
                                       else s_all)[:, t0:t0 + 1],
                        )

                    def tile_epilogue(i):
                        t0 = tcix * tpc + i
                        h_t = h_ts[i]
                        nc.vector.tensor_tensor(
                            out=s_all[:, t0:t0 + 1],
                            in0=s_all[:, t0:t0 + 1],
                            in1=s_c[:, t0:t0 + 1],
                            op=mybir.AluOpType.add,
                        )
                        nc.vector.tensor_scalar_min(
                            s_c[:, t0:t0 + 1], s_all[:, t0:t0 + 1], 60.0)
                        nc.scalar.activation(
                            out=e_f32[:, t0:t0 + 1], in_=s_c[:, t0:t0 + 1],
                            func=mybir.ActivationFunctionType.Exp,
                        )
                        if t0 >= nt - 2:
                            # f32r exp copy only needed by the PE-path
                            # weighted sums of the final two tiles
                            nc.vector.tensor_copy(e_r[:, t0:t0 + 1],
                                                  e_f32[:, t0:t0 + 1])
                        # queue this tile's weighted-sum fold; emit the
                        # previous tile's now (one-tile pipeline slack so
                        # no engine waits on the scores->exp chain)
                        wsum_pending.append((t0, h_t))
                        if len(wsum_pending) > 1:
                            emit_wacc(*wsum_pending.pop(0))

                    for i in range(tpc):
                        h_ts[i] = hpool.tile([128, units], F32R,
                                             tag="H", name="h_t")

                    if first_chunk and nuh == 2:
                        # uh0: sequential full sweeps, arrival-paced by the
                        # interleaved k-quarter DMAs.  The DMA bus is
                        # saturated here, so fixed dummy-matmul fillers
                        # between sweeps soak up the deterministic arrival
                        # lag instead of idling the PE.
                        for i in range(tpc):
                            ps = mainps.tile([128, uh_w], F32, tag="ps",
                                             name="ps")
                            half_matmuls(i, 0, range(nk), ps)
                            half_epilogue(i, 0, ps)
                            if i < tpc - 1:
                                for _ in range(n_fill):
                                    nc.tensor.matmul(
                                        out=o_ps[0:1, 0:uh_w],
                                        lhsT=warm[:, 0:1],
                                        rhs=warm[:, 0:uh_w],
                                        start=True, stop=True,
                                    )
                        # uh1: k-quarter-interleaved across the 4 tiles so
                        # each quarter's arrival feeds a full PE group
                        # instead of gating whole-tile sweeps (4 open PSUM
                        # accumulations — exactly the mainps pool).
                        ps_u1 = {}
                        for q in range(0, nk, kq):
                            for i in range(tpc):
                                if i not in ps_u1:
                                    ps_u1[i] = mainps.tile(
                                        [128, uh_w], F32, tag="ps",
                                        name="ps")
                                half_matmuls(i, 1, range(q, q + kq),
                                             ps_u1[i])
                        for i in range(tpc):
                            half_epilogue(i, 1, ps_u1[i])
                            tile_epilogue(i)
                    else:
                        for i in range(tpc):
                            for uh in range(nuh):
                                ps = mainps.tile([128, uh_w], F32,
                                                 tag="ps", name="ps")
                                half_matmuls(i, uh, range(nk), ps)
                                half_epilogue(i, uh, ps)
                            tile_epilogue(i)

                # ---- exp row-sums first: they only need the exps, so the
                # reduce + DMA overlap the trailing weighted-sum matmuls
                esum = spool.tile([128, 1], F32, tag="esum", name="esum")
                nc.vector.reduce_sum(out=esum[:, :], in_=e_f32[:, :],
                                     axis=mybir.AxisListType.X)
                nc.sync.dma_start(out=esums[b], in_=esum[:, :])

                for item in wsum_pending:
                    emit_wacc(*item)
                wsum_pending = []

                # ---- ship the raw weighted sum; the PSUM->SBUF copy is
                # split across scalar+vector so the halves run in parallel
                o_sb = scratch.tile([1, units], F32, tag="o_sb", name="o_sb")
                nc.scalar.copy(o_sb[0:1, 0:uh_w], o_ps[0:1, 0:uh_w])
                nc.vector.tensor_copy(o_sb[0:1, uh_w:units],
                                      o_ps[0:1, uh_w:units])
                nc.sync.dma_start(out=o_raw[b:b + 1, :], in_=o_sb[:, :])

    nc.finalize()
    return nc


def _prep_inputs(sequences1, sequences2, W1_kernel, W1_bias, W2_kernel,
                 W2_bias, W_kernel, W_bias, context_vector):
    """Host-side folding + layout. Returns (per-core in_maps, beff)."""
    U = UNITS
    W = np.asarray(W_kernel, np.float32)
    M1 = np.asarray(W1_kernel, np.float32) @ W[:U]
    M2 = np.asarray(W2_kernel, np.float32) @ W[U:]
    beff = (np.asarray(W1_bias, np.float32) @ W[:U]
            + np.asarray(W2_bias, np.float32) @ W[U:]
            + np.asarray(W_bias, np.float32))

    M = np.concatenate([M1, M2], axis=0)                   # [KD, U]
    mw = np.ascontiguousarray(M.reshape(KD // 128, 128, U), np.float32)
    c = np.asarray(context_vector, np.float32).reshape(U)
    crep = np.ascontiguousarray(np.broadcast_to(c, (128, U)), np.float32)

    x1 = np.asarray(sequences1, np.float32)
    x2 = np.asarray(sequences2, np.float32)
    has_bias = bool(np.any(beff != 0.0))
    brep = np.ascontiguousarray(np.broadcast_to(beff, (128, U)), np.float32)
    in_maps = []
    for core in range(N_CORES):
        bs = slice(core * BPC, (core + 1) * BPC)
        xcat = np.concatenate([x1[bs], x2[bs]], axis=2)    # [BPC, T, KD]
        # -> [BPC, KD/128, 128, T]: xt[b, k, p, t] = xcat[b, t, 128k + p]
        xtc = np.ascontiguousarray(
            xcat.transpose(0, 2, 1).reshape(BPC, KD // 128, 128, T)
        )
        im = {"xt": xtc, "mw": mw, "crep": crep}
        if has_bias:
            im["brep"] = brep
        in_maps.append(im)
    return in_maps, has_bias


def kernel(sequences1, sequences2, W1_kernel, W1_bias, W2_kernel, W2_bias,
           W_kernel, W_bias, context_vector):
    in_maps, has_bias = _prep_inputs(
        sequences1, sequences2, W1_kernel, W1_bias, W2_kernel, W2_bias,
        W_kernel, W_bias, context_vector)
    key = ("full", has_bias)
    if key not in _NC_CACHE:
        _NC_CACHE[key] = build_nc(has_bias=has_bias)
    nc = _NC_CACHE[key]
    res = run_bass_kernel_spmd(nc, in_maps, list(range(N_CORES)))
    outs = []
    for r in res.results:
        z = r["esums"].reshape(BPC, 128).sum(axis=1, dtype=np.float64)
        outs.append(r["o_raw"] / z[:, None].astype(np.float32))
    return np.concatenate(outs, axis=0).astype(np.float32)


# revision 49
# speedup vs baseline: 1.0197x; 1.0197x over previous
"""TRN2 Bass kernel for nn_BimodalAttention.

Reference computation (B=16, T=2048, D1=D2=1024, U=1024):
    f1 = X1 @ W1 + b1 ; f2 = X2 @ W2 + b2
    H  = tanh(concat(f1, f2) @ W + b)            # [B,T,U]
    s  = H @ c ; a = softmax(s, axis=T)          # [B,T,1]
    out[b] = sum_t a[b,t] * H[b,t]               # [B,U]

Device strategy (data-parallel over batch, 2 batches per core, 8 cores):
  * Host folds the linear chain: M1 = W1 @ W[:U], M2 = W2 @ W[U:], so the
    device computes H = tanh(Xcat @ M + beff) with M = [M1; M2] — half the
    matmul FLOPs of the literal graph.
  * Host pre-transposes/tiles Xcat to [B, K/128, 128, T] so every lhsT tile
    DMA is contiguous, and replicates the context vector across the 128
    partitions so scores are row-local DVE work.
  * Main matmuls run as float32r (full PE rate; measured steady cadence
    ~227ns per 512-col matmul = stream time + ~15ns, i.e. at the hw
    floor).  tanh writes H as f32r directly (the ACT engine rounds at
    write), so the weighted-sum matmuls consume it with no cast; the
    score path reads the same bytes through an f32 bitcast.
  * PE warm-up + deterministic dummy fillers (on memset data, no DMA
    dependency) cover the ~8us DMA queue-startup window and the
    DMA-saturated weight-load phase; chunk 0's uh1 sweeps are k-quarter
    interleaved across the 4 t-tiles (4 open PSUM accumulations) so each
    weight quarter's arrival feeds a full PE group.
  * Softmax over T: no max-subtraction (scores are ~N(0,11) by
    construction; exp overflows only past 88) — a clamp at 60 guards
    against inf.  The weighted time-sum uses the unnormalized exp weights
    (PE matmuls, 1-tile pipeline slack); normalization by 1/Z happens on
    the HOST (kernel ships the raw weighted sum and the per-partition exp
    sums), which shortens the device tail and frees PSUM banks so both
    batches' o_ps can overlap.
"""
import numpy as np

import concourse.bacc as bacc
import concourse.mybir as mybir
from concourse.bass_utils import run_bass_kernel_spmd
from concourse.tile import TileContext

F32 = mybir.dt.float32
F32R = mybir.dt.float32r

N_CORES = 8
B, T, D, UNITS = 16, 2048, 1024, 1024
KD = 2 * D          # folded contraction dim (seq1 ++ seq2)
BPC = B // N_CORES  # batches per core

_NC_CACHE = {}


def build_nc(bpc=BPC, t=T, kd=KD, units=UNITS, has_bias=False, tchunk=512,
             n_warm=44, n_fill=6):
    """Build the per-core Bass module (same program on all cores)."""
    nc = bacc.Bacc(None, target_bir_lowering=False)

    nk = kd // 128              # k-blocks in contraction
    nt = t // 128               # t-blocks
    nuh = (units + 511) // 512  # 512-wide u column groups
    uh_w = units // nuh
    ntc = t // tchunk           # streamed X chunks per batch
    tpc = tchunk // 128         # t-blocks per chunk

    xt = nc.declare_dram_parameter("xt", [bpc, nk, 128, t], F32R, isOutput=False)
    mw = nc.declare_dram_parameter("mw", [nk, 128, units], F32R, isOutput=False)
    crep = nc.declare_dram_parameter("crep", [128, units], F32, isOutput=False)
    if has_bias:
        brep = nc.declare_dram_parameter("brep", [128, units], F32,
                                         isOutput=False)
    o_raw = nc.declare_dram_parameter("o_raw", [bpc, units], F32, isOutput=True)
    esums = nc.declare_dram_parameter("esums", [bpc, 128, 1], F32,
                                      isOutput=True)

    with TileContext(nc) as tc:
        with (
            tc.tile_pool(name="wpool", bufs=1) as wpool,
            tc.tile_pool(name="xpool", bufs=2) as xpool,
            tc.tile_pool(name="hpool", bufs=tpc + 2) as hpool,
            tc.tile_pool(name="spool", bufs=2) as spool,
            tc.tile_pool(name="scratch", bufs=2) as scratch,
            tc.tile_pool(name="mainps", bufs=4, space="PSUM") as mainps,
            tc.tile_pool(name="outps", bufs=2, space="PSUM") as outps,
        ):
            # ---- resident small tensors -------------------------------
            mwt = wpool.tile([128, nk * units], F32R, name="mwt")
            mwt4 = mwt.rearrange("p (k h u) -> p k h u", k=nk, h=nuh)
            kq = max(1, nk // 4)
            mw_r = mw.rearrange("k p u -> p k u")
            crep_s = wpool.tile([128, units], F32, name="crep_s")
            if has_bias:
                brep_s = wpool.tile([128, units], F32, name="brep_s")
                nc.sync.dma_start(out=brep_s[:, :], in_=brep[:, :])

            # ones column (f32r) for the final cross-partition reduction
            # of the ACT/DVE-accumulated weighted sum
            ones_f = wpool.tile([128, 1], F32, name="ones_f")
            ones_r = wpool.tile([128, 1], F32R, name="ones_r")
            nc.vector.memset(ones_f[:, :], 1.0)
            nc.vector.tensor_copy(ones_r[:, :], ones_f[:, :])

            first_deferred = True
            for b in range(bpc):
                s_all = spool.tile([128, nt], F32, tag="s_all", name="s_all")
                s_c = spool.tile([128, nt], F32, tag="s_c", name="s_c")
                e_f32 = spool.tile([128, nt], F32, tag="e_f32", name="e_f32")
                e_r = spool.tile([128, nt], F32R, tag="e_r", name="e_r")
                o_ps = outps.tile([1, units], F32, tag="o_ps", name="o_ps")
                # weighted-sum accumulator: tiles 0..nt-3 are accumulated
                # OFF the PE (ACT per-partition scale-mul + DVE add), so
                # only the last two tiles' weighted sums plus one final
                # ones-vector reduction run as matmuls -> ~7us less
                # PE-critical-path work per batch.  The last 'acc' tile's
                # add writes the f32r copy the final matmul needs.
                acc0 = spool.tile([128, units], F32, tag="acc", name="acc0")
                acc1 = spool.tile([128, units], F32, tag="acc", name="acc1")
                accs = [acc0, acc1]
                acc_r = spool.tile([128, units], F32R, tag="acc_r",
                                   name="acc_r", bufs=1)
                wsum_pending = []

                def emit_wacc(t0, h_t):
                    """Fold tile t0 into the weighted sum: one fused DVE
                    op acc' = (H * e_col) + acc (ping-pong accumulators,
                    no same-instruction aliasing)."""
                    e_col = e_f32[:, t0:t0 + 1]
                    h_f = h_t.bitcast(F32)
                    if t0 >= nt - 2:
                        # last two tiles: PE path (keeps the end-of-batch
                        # chain short: exp -> 2 matmuls).  Tile nt-2 OPENS
                        # the o_ps accumulation group (its e is ready well
                        # before the DVE acc_r copy finishes); the
                        # cross-partition acc reduction follows it so the
                        # PE never stalls on the copy.
                        for wuh in range(nuh):
                            nc.tensor.matmul(
                                out=o_ps[0:1, wuh * uh_w:(wuh + 1) * uh_w],
                                lhsT=e_r[:, t0:t0 + 1],
                                rhs=h_t[:, wuh * uh_w:(wuh + 1) * uh_w],
                                start=(t0 == nt - 2),
                                stop=(t0 == nt - 1),
                            )
                        if t0 == nt - 2:
                            for wuh in range(nuh):
                                nc.tensor.matmul(
                                    out=o_ps[0:1,
                                             wuh * uh_w:(wuh + 1) * uh_w],
                                    lhsT=ones_r[:, 0:1],
                                    rhs=acc_r[:, wuh * uh_w:(wuh + 1) * uh_w],
                                    start=False, stop=False,
                                )
                        return
                    dst, src = accs[t0 % 2], accs[1 - t0 % 2]
                    if t0 == 0:
                        nc.vector.tensor_scalar_mul(dst[:, :], h_f[:, :],
                                                    e_col)
                    else:
                        nc.vector.scalar_tensor_tensor(
                            out=dst[:, :], in0=h_f[:, :], scalar=e_col,
                            in1=src[:, :],
                            op0=mybir.AluOpType.mult,
                            op1=mybir.AluOpType.add,
                        )
                    if t0 == nt - 3:
                        nc.vector.tensor_copy(acc_r[:, :], dst[:, :])

                if b == 0:
                    # PE warm-up: dummy matmuls into o_ps (the real t0=0
                    # weighted-sum matmul re-clears it with start=True).
                    # Gets HAM to K=8/8 while the first loads stream in.
                    # The warm data is produced on-chip (memset + DVE cast
                    # - a legal f32r producer) so the warm-up has no DMA
                    # dependency and starts ~1us in, covering the DMA
                    # queue-startup dead window.
                    warm_f = wpool.tile([128, uh_w], F32, name="warm_f")
                    warm = wpool.tile([128, uh_w], F32R, name="warm")
                    nc.vector.memset(warm_f[:, :], 0.0)
                    nc.vector.tensor_copy(warm[:, :], warm_f[:, :])
                    for _ in range(n_warm):
                        nc.tensor.matmul(
                            out=o_ps[0:1, 0:uh_w],
                            lhsT=warm[:, 0:1], rhs=warm[:, 0:uh_w],
                            start=True, stop=True,
                        )

                for tcix in range(ntc):
                    first_chunk = first_deferred
                    a_t = xpool.tile([128, nk * tchunk], F32R, tag="a_t",
                                     name="a_t")
                    a_t3 = a_t.rearrange("p (k w) -> p k w", k=nk)
                    x_src = xt[b].rearrange("k p w -> p k w")[
                        :, :, tcix * tchunk:(tcix + 1) * tchunk]
                    if first_chunk:
                        # context vector first (the score chain needs it
                        # from the very first tile), then interleaved
                        # k-quarter loads of mw[uh0] and chunk 0 so the
                        # uh0 tiles start after ~2MB; uh1 quarters follow
                        # (consumed by the later uh1 pass of chunk 0).
                        nc.sync.dma_start(out=crep_s[:, :], in_=crep[:, :])
                        for q in range(0, nk, kq):
                            nc.sync.dma_start(
                                out=mwt4[:, q:q + kq, 0, :],
                                in_=mw_r[:, q:q + kq, 0:uh_w],
                            )
                            nc.sync.dma_start(
                                out=a_t3[:, q:q + kq, :],
                                in_=x_src[:, q:q + kq, :],
                            )
                        for uh in range(1, nuh):
                            for q in range(0, nk, kq):
                                nc.sync.dma_start(
                                    out=mwt4[:, q:q + kq, uh, :],
                                    in_=mw_r[:, q:q + kq,
                                             uh * uh_w:(uh + 1) * uh_w],
                                )
                        first_deferred = False
                    else:
                        # halves: finer arrival granularity so the first
                        # tiles of a late chunk start on the first half
                        half = tchunk // 2
                        nc.sync.dma_start(out=a_t3[:, :, 0:half],
                                          in_=x_src[:, :, 0:half])
                        nc.sync.dma_start(out=a_t3[:, :, half:],
                                          in_=x_src[:, :, half:])

                    h_ts = {}

                    def half_matmuls(i, uh, ks, ps):
                        for k in ks:
                            nc.tensor.matmul(
                                out=ps[:, :],
                                lhsT=a_t[:, k * tchunk + i * 128:
                                         k * tchunk + (i + 1) * 128],
                                rhs=mwt[:, k * units + uh * uh_w:
                                        k * units + (uh + 1) * uh_w],
                                start=(k == 0),
                                stop=(k == nk - 1),
                            )

                    def half_epilogue(i, uh, ps):
                        t0 = tcix * tpc + i
                        h_t = h_ts[i]
                        uhs = slice(uh * uh_w, (uh + 1) * uh_w)
                        if has_bias:
                            nc.vector.tensor_tensor(
                                out=ps[:, :], in0=ps[:, :],
                                in1=brep_s[:, uhs],
                                op=mybir.AluOpType.add,
                            )
                        # tanh writes the f32r H half directly (the ACT
                        # engine rounds at write) — no separate cast
                        nc.scalar.activation(
                            out=h_t[:, uhs],
                            in_=ps[:, :],
                            func=mybir.ActivationFunctionType.Tanh,
                        )
                        # score partial in ONE fused DVE op:
                        # junk = (h*1.0) \odot c, accum_out = row-sum
                        junk = scratch.tile([128, uh_w], F32,
                                            tag="junk", name="junk",
                                            bufs=3)
                        nc.vector.scalar_tensor_tensor(
                            out=junk[:, :],
                            in0=h_t.bitcast(F32)[:, uhs],
                            scalar=1.0,
                            in1=crep_s[:, uhs],
                            op0=mybir.AluOpType.mult,
                            op1=mybir.AluOpType.mult,
                            accum_out=(s_c if uh == 0
                                       else s_all)[:, t0:t0 + 1],
                        )

                    def tile_epilogue(i):
                        t0 = tcix * tpc + i
                        h_t = h_ts[i]
                        nc.vector.tensor_tensor(
                            out=s_all[:, t0:t0 + 1],
                            in0=s_all[:, t0:t0 + 1],
                            in1=s_c[:, t0:t0 + 1],
                            op=mybir.AluOpType.add,
                        )
                        nc.vector.tensor_scalar_min(
                            s_c[:, t0:t0 + 1], s_all[:, t0:t0 + 1], 60.0)
                        nc.scalar.activation(
                            out=e_f32[:, t0:t0 + 1], in_=s_c[:, t0:t0 + 1],
                            func=mybir.ActivationFunctionType.Exp,
                        )
                        if t0 >= nt - 2:
                            # f32r exp copy only needed by the PE-path
                            # weighted sums of the final two tiles
                            nc.vector.tensor_copy(e_r[:, t0:t0 + 1],
                                                  e_f32[:, t0:t0 + 1])
                        # queue this tile's weighted-sum fold; emit the
                        # previous tile's now (one-tile pipeline slack so
                        # no engine waits on the scores->exp chain)
                        wsum_pending.append((t0, h_t))
                        if len(wsum_pending) > 1:
                            emit_wacc(*wsum_pending.pop(0))

                    for i in range(tpc):
                        h_ts[i] = hpool.tile([128, units], F32R,
                                             tag="H", name="h_t")

                    if first_chunk and nuh == 2:
                        # uh0: sequential full sweeps, arrival-paced by the
                        # interleaved k-quarter DMAs.  The DMA bus is
                        # saturated here, so fixed dummy-matmul fillers
                        # between sweeps soak up the deterministic arrival
                        # lag instead of idling the PE.
                        for i in range(tpc):
                            ps = mainps.tile([128, uh_w], F32, tag="ps",
                                             name="ps")
                            half_matmuls(i, 0, range(nk), ps)
                            half_epilogue(i, 0, ps)
                            if i < tpc - 1:
                                for _ in range(n_fill):
                                    nc.tensor.matmul(
                                        out=o_ps[0:1, 0:uh_w],
                                        lhsT=warm[:, 0:1],
                                        rhs=warm[:, 0:uh_w],
                                        start=True, stop=True,
                                    )
                        # uh1: k-quarter-interleaved across the 4 tiles so
                        # each quarter's arrival feeds a full PE group
                        # instead of gating whole-tile sweeps (4 open PSUM
                        # accumulations — exactly the mainps pool).
                        ps_u1 = {}
                        for q in range(0, nk, kq):
                            for i in range(tpc):
                                if i not in ps_u1:
                                    ps_u1[i] = mainps.tile(
                                        [128, uh_w], F32, tag="ps",
                                        name="ps")
                                half_matmuls(i, 1, range(q, q + kq),
                                             ps_u1[i])
                        for i in range(tpc):
                            half_epilogue(i, 1, ps_u1[i])
                            tile_epilogue(i)
                    else:
                        for i in range(tpc):
                            for uh in range(nuh):
                                ps = mainps.tile([128, uh_w], F32,
                                                 tag="ps", name="ps")
                                half_matmuls(i, uh, range(nk), ps)
                                half_epilogue(i, uh, ps)
                            tile_epilogue(i)

                # ---- exp row-sums first: they only need the exps, so the
                # reduce + DMA overlap the trailing weighted-sum matmuls
                esum = spool.tile([128, 1], F32, tag="esum", name="esum")
                nc.vector.reduce_sum(out=esum[:, :], in_=e_f32[:, :],
                                     axis=mybir.AxisListType.X)
                nc.sync.dma_start(out=esums[b], in_=esum[:, :])

                for item in wsum_pending:
                    emit_wacc(*item)
                wsum_pending = []

                # ---- ship the raw weighted sum; the PSUM->SBUF copy is
                # split across scalar+vector so the halves run in parallel
                o_sb = scratch.tile([1, units], F32, tag="o_sb", name="o_sb")
                nc.scalar.copy(o_sb[0:1, 0:uh_w], o_ps[0:1, 0:uh_w])
                nc.vector.tensor_copy(o_sb[0:1, uh_w:units],
                                      o_ps[0:1, uh_w:units])
                nc.sync.dma_start(out=o_raw[b:b + 1, :], in_=o_sb[:, :])

    nc.finalize()
    return nc


def _prep_inputs(sequences1, sequences2, W1_kernel, W1_bias, W2_kernel,
                 W2_bias, W_kernel, W_bias, context_vector):
    """Host-side folding + layout. Returns (per-core in_maps, beff)."""
    U = UNITS
    W = np.asarray(W_kernel, np.float32)
    M1 = np.asarray(W1_kernel, np.float32) @ W[:U]
    M2 = np.asarray(W2_kernel, np.float32) @ W[U:]
    beff = (np.asarray(W1_bias, np.float32) @ W[:U]
            + np.asarray(W2_bias, np.float32) @ W[U:]
            + np.asarray(W_bias, np.float32))

    M = np.concatenate([M1, M2], axis=0)                   # [KD, U]
    mw = np.ascontiguousarray(M.reshape(KD // 128, 128, U), np.float32)
    c = np.asarray(context_vector, np.float32).reshape(U)
    crep = np.ascontiguousarray(np.broadcast_to(c, (128, U)), np.float32)

    x1 = np.asarray(sequences1, np.float32)
    x2 = np.asarray(sequences2, np.float32)
    has_bias = bool(np.any(beff != 0.0))
    brep = np.ascontiguousarray(np.broadcast_to(beff, (128, U)), np.float32)
    in_maps = []
    for core in range(N_CORES):
        bs = slice(core * BPC, (core + 1) * BPC)
        xcat = np.concatenate([x1[bs], x2[bs]], axis=2)    # [BPC, T, KD]
        # -> [BPC, KD/128, 128, T]: xt[b, k, p, t] = xcat[b, t, 128k + p]
        xtc = np.ascontiguousarray(
            xcat.transpose(0, 2, 1).reshape(BPC, KD // 128, 128, T)
        )
        im = {"xt": xtc, "mw": mw, "crep": crep}
        if has_bias:
            im["brep"] = brep
        in_maps.append(im)
    return in_maps, has_bias


def kernel(sequences1, sequences2, W1_kernel, W1_bias, W2_kernel, W2_bias,
           W_kernel, W_bias, context_vector):
    in_maps, has_bias = _prep_inputs(
        sequences1, sequences2, W1_kernel, W1_bias, W2_kernel, W2_bias,
        W_kernel, W_bias, context_vector)
    key = ("full", has_bias)
    if key not in _NC_CACHE:
        _NC_CACHE[key] = build_nc(has_bias=has_bias)
    nc = _NC_CACHE[key]
    res = run_bass_kernel_spmd(nc, in_maps, list(range(N_CORES)))
    outs = []
    for r in res.results:
        z = r["esums"].reshape(BPC, 128).sum(axis=1, dtype=np.float64)
        outs.append(r["o_raw"] / z[:, None].astype(np.float32))
    return np.concatenate(outs, axis=0).astype(np.float32)
